# revision 7
# baseline (speedup 1.0000x reference)
"""Trainium2 Bass kernel for 3D neighborhood attention (NATTEN-style).

Sharding: H axis (32) split across 8 cores, 4 own rows + a 2-row halo each
side (host-padded to a uniform 8-row window; W padded by 2 each side). All
neighborhood masking is folded into the score matmul contraction via
indicator/penalty rows:
  scoresT[k,q] = k.T q + sum_r U[r,k] * V[r,q]
with U = key-side (d,h,w)-position indicators (0/1) and V = query-side
-30000 * (1 - valid) penalties, so exp() zeroes out-of-window keys with no
extra vector ops. Contraction K = 64 (head dim) + 4 + 8 + 52 = 128 exactly.

Layouts (chosen so every matmul operand is a contiguous SBUF slice):
  kT per head: [128, NV], free dim w-major: pos = w*32 + d*8 + h
  qT per head: [128, NQ], free dim wtile-major: pos = wt*256 + d*64 + h*16 + wl
  key chunk (wt, ci) = kT cols [512*wt + 128*ci, +128)  (w' quad, all d', h')
  query block wt     = qT cols [256*wt, +256)

Pipeline per core (one NEFF): QKV (f32r) -> scoresT (bf16, keys-major) ->
exp (ACT, masked keys -> 0) -> attn.T@[v|1] (bf16; col 64 of the rhs is ones,
giving softmax sums alongside the unnormalized output) -> reciprocal+scale
(DVE) -> PE transpose -> proj (bf16) -> strided DMA out.
"""
import numpy as np
import ml_dtypes

import concourse.bass as bass
import concourse.bacc as bacc
import concourse.mybir as mybir
from concourse.tile import TileContext
from concourse.bass_utils import run_bass_kernel_spmd

F32R = mybir.dt.float32r
BF16 = mybir.dt.bfloat16
F32 = mybir.dt.float32

NCORES = 8
D, H, W, C = 4, 32, 48, 256
HEADS, HD = 4, 64
KD, KH, KW = 3, 5, 5
SCALE = HD ** -0.5
BIG = 30000.0

HH = 8            # halo rows per core (padded-uniform)
WP = W + 4        # padded W
NV = D * HH * WP  # 1664 voxels per shard (with halo+pad)
NQ = D * 4 * W    # 768 own queries per core
NWT = 3           # w tiles of 16 queries
NCK = 5           # key chunks of 128 per w-tile (4 w' x 4 d x 8 h)

_CACHE = {}


def _build_program():
    nc = bacc.Bacc("TRN2", target_bir_lowering=False, debug=False,
                   num_devices=NCORES)
    xT_in = nc.declare_dram_parameter("xT", [C, NV], F32R, isOutput=False)
    xTq_in = nc.declare_dram_parameter("xTq", [C, NQ], F32R, isOutput=False)
    wq_in = nc.declare_dram_parameter("wq", [C, C], F32R, isOutput=False)
    wk_in = nc.declare_dram_parameter("wk", [C, C], F32R, isOutput=False)
    wv_in = nc.declare_dram_parameter("wv", [C, C], F32R, isOutput=False)
    wp_in = nc.declare_dram_parameter("wp", [C, C], BF16, isOutput=False)
    u_in = nc.declare_dram_parameter("u", [64, NV], BF16, isOutput=False)
    v_in = nc.declare_dram_parameter("vq", [64, NQ], BF16, isOutput=False)
    bqkv_in = nc.declare_dram_parameter("bqkv", [128, 6], F32, isOutput=False)
    bv_in = nc.declare_dram_parameter("bv", [1, C], F32, isOutput=False)
    bp_in = nc.declare_dram_parameter("bp", [1, C], F32, isOutput=False)
    ident_in = nc.declare_dram_parameter("ident", [128, 128], BF16, isOutput=False)
    y_out = nc.declare_dram_parameter("y", [NQ, C], F32, isOutput=True)
    v_dram = nc.dram_tensor("v_scratch", [NV, C], BF16)

    with TileContext(nc) as tc:
        with (
            tc.tile_pool(name="const", bufs=1) as cp,
            tc.tile_pool(name="psA", bufs=2, space="PSUM") as psA,
            tc.tile_pool(name="psS", bufs=2, space="PSUM") as psS,
            tc.tile_pool(name="work", bufs=3) as wkp,
        ):
            # ---- constant / input loads ----
            xT = [cp.tile([128, NV], F32R, tag=f"xT{i}", name=f"xT{i}")
                  for i in range(2)]
            xTq = [cp.tile([128, NQ], F32R, tag=f"xTq{i}", name=f"xTq{i}")
                   for i in range(2)]
            for i in range(2):
                nc.sync.dma_start(out=xT[i][:], in_=xT_in[128 * i:128 * (i + 1), :])
                nc.sync.dma_start(out=xTq[i][:], in_=xTq_in[128 * i:128 * (i + 1), :])
            wq_t = [cp.tile([128, C], F32R, tag=f"wq{i}", name=f"wq{i}") for i in range(2)]
            wk_t = [cp.tile([128, C], F32R, tag=f"wk{i}", name=f"wk{i}") for i in range(2)]
            wv_t = [cp.tile([128, C], F32R, tag=f"wv{i}", name=f"wv{i}") for i in range(2)]
            for i in range(2):
                nc.sync.dma_start(out=wq_t[i][:], in_=wq_in[128 * i:128 * (i + 1), :])
                nc.sync.dma_start(out=wk_t[i][:], in_=wk_in[128 * i:128 * (i + 1), :])
                nc.sync.dma_start(out=wv_t[i][:], in_=wv_in[128 * i:128 * (i + 1), :])
            wp_t = [cp.tile([128, C], BF16, tag=f"wp{i}", name=f"wp{i}") for i in range(2)]
            for i in range(2):
                nc.sync.dma_start(out=wp_t[i][:], in_=wp_in[128 * i:128 * (i + 1), :])
            kT = [cp.tile([128, NV], BF16, tag=f"kT{h}", name=f"kTh{h}")
                  for h in range(HEADS)]
            qT = [cp.tile([128, NQ], BF16, tag=f"qT{h}", name=f"qTh{h}")
                  for h in range(HEADS)]
            for h in range(HEADS):
                nc.sync.dma_start(out=kT[h][64:128, :], in_=u_in[:])
                nc.sync.dma_start(out=qT[h][64:128, :], in_=v_in[:])
            bqkv = cp.tile([128, 6], F32)
            nc.sync.dma_start(out=bqkv[:], in_=bqkv_in[:])
            bv_row = cp.tile([1, C], F32)
            bp_row = cp.tile([1, C], F32)
            nc.sync.dma_start(out=bv_row[:], in_=bv_in[:])
            nc.sync.dma_start(out=bp_row[:], in_=bp_in[:])
            bv_b = cp.tile([128, C], F32)
            bp_b = cp.tile([128, C], F32)
            nc.gpsimd.partition_broadcast(bv_b[:], bv_row[:])
            nc.gpsimd.partition_broadcast(bp_b[:], bp_row[:])
            ident = cp.tile([128, 128], BF16)
            nc.sync.dma_start(out=ident[:], in_=ident_in[:])
            v_g = [cp.tile([128, 260], BF16, tag=f"vg{i}", name=f"vg{i}")
                   for i in range(NWT * NCK)]
            for t in v_g:
                nc.vector.memset(t[:], 0.0)
                for h in range(HEADS):
                    nc.vector.memset(t[:, 65 * h + 64:65 * h + 65], 1.0)

            # ---- QKV ----
            for m in range(2):          # output channel chunk (= head pair)
                # q: own voxels, from the query-ordered xTq copy (contiguous)
                for nn in range(2):
                    ps = psA.tile([128, 384], F32, tag="ps")
                    for kc in range(2):
                        nc.tensor.matmul(ps[:], wq_t[kc][:, 128 * m:128 * (m + 1)],
                                         xTq[kc][:, 384 * nn:384 * (nn + 1)],
                                         start=(kc == 0), stop=(kc == 1))
                    for hh in range(2):
                        nc.vector.tensor_scalar(
                            out=qT[2 * m + hh][0:64, 384 * nn:384 * (nn + 1)],
                            in0=ps[64 * hh:64 * (hh + 1), :],
                            scalar1=bqkv[64 * hh:64 * (hh + 1), m:m + 1],
                            scalar2=None, op0=mybir.AluOpType.add)
                # k: all voxels; copyback scatters (h,w)-order into w-major kT
                for nn in range(4):     # d plane (vox quarter, 416 each)
                    ps = psA.tile([128, 416], F32, tag="ps")
                    for kc in range(2):
                        nc.tensor.matmul(
                            ps[:], wk_t[kc][:, 128 * m:128 * (m + 1)],
                            xT[kc][:, 416 * nn:416 * (nn + 1)],
                            start=(kc == 0), stop=(kc == 1))
                    for hh in range(2):
                        dst = kT[2 * m + hh][0:64, :].rearrange(
                            "p (w d h) -> p h w d", w=WP, d=D, h=HH)[:, :, :, nn]
                        src = ps[64 * hh:64 * (hh + 1), :].rearrange(
                            "p (h w) -> p h w", h=HH, w=WP)
                        nc.vector.tensor_scalar(
                            out=dst, in0=src,
                            scalar1=bqkv[64 * hh:64 * (hh + 1), 2 + m:3 + m],
                            scalar2=None, op0=mybir.AluOpType.add)
            # v (vox-major, all voxels) -> DRAM scratch
            for t in range(NV // 128):
                ps = psA.tile([128, C], F32, tag="ps")
                for kc in range(2):
                    nc.tensor.matmul(ps[:], xT[kc][:, 128 * t:128 * (t + 1)],
                                     wv_t[kc][:], start=(kc == 0), stop=(kc == 1))
                vsb = wkp.tile([128, C], BF16, tag="vsb")
                nc.vector.tensor_tensor(out=vsb[:], in0=ps[:], in1=bv_b[:],
                                        op=mybir.AluOpType.add)
                nc.sync.dma_start(out=v_dram[128 * t:128 * (t + 1), :], in_=vsb[:])

            # ---- gather v into key-chunk order (w', d, h) ----
            vv = v_dram.ap().rearrange("(d h w) c -> w d h c", d=D, h=HH, w=WP)
            for wt in range(NWT):
                for ci in range(NCK):
                    dst = v_g[wt * NCK + ci]
                    lo = 16 * wt + 4 * ci
                    for h in range(HEADS):
                        nc.sync.dma_start(
                            out=dst[:, 65 * h:65 * h + 64],
                            in_=vv[lo:lo + 4, :, :, 64 * h:64 * (h + 1)])

            # ---- attention ----
            ao = [cp.tile([128, C], BF16, tag=f"ao{i}", name=f"ao{i}")
                  for i in range(6)]
            for wt in range(NWT):
                for h in range(HEADS):
                    ps_s = psS.tile([128, 1280], F32, tag="ps_s")
                    for ci in range(NCK):
                        nc.tensor.matmul(
                            ps_s[:, 256 * ci:256 * (ci + 1)],
                            kT[h][:, 512 * wt + 128 * ci:512 * wt + 128 * (ci + 1)],
                            qT[h][:, 256 * wt:256 * (wt + 1)],
                            start=True, stop=True)
                    ex = wkp.tile([128, 1280], BF16, tag="ex")
                    nc.scalar.activation(ex[:], ps_s[:],
                                         mybir.ActivationFunctionType.Exp)
                    for s in range(2):
                        ps_o = psA.tile([128, 65], F32, tag="ps")
                        for ci in range(NCK):
                            nc.tensor.matmul(
                                ps_o[:],
                                ex[:, 256 * ci + 128 * s:256 * ci + 128 * (s + 1)],
                                v_g[wt * NCK + ci][:, 65 * h:65 * (h + 1)],
                                start=(ci == 0), stop=(ci == NCK - 1))
                        rt = wkp.tile([128, 1], F32, tag="rt")
                        nc.vector.reciprocal(rt[:], ps_o[:, 64:65])
                        nc.vector.tensor_scalar(
                            out=ao[2 * wt + s][:, 64 * h:64 * (h + 1)],
                            in0=ps_o[:, 0:64], scalar1=rt[:], scalar2=None,
                            op0=mybir.AluOpType.mult)

            # ---- transpose + proj + out ----
            yv = y_out.ap().rearrange("(wt d h wl) c -> wt d h wl c",
                                      wt=NWT, d=D, h=4, wl=16)
            for b in range(6):
                wt, s = b // 2, b % 2
                ps_t = psA.tile([128, 256], BF16, tag="ps")
                for u in range(2):
                    nc.tensor.transpose(ps_t[:, 128 * u:128 * (u + 1)],
                                        ao[b][:, 128 * u:128 * (u + 1)], ident[:])
                aoT = wkp.tile([128, 256], BF16, tag="aoT")
                nc.vector.tensor_copy(aoT[:], ps_t[:])
                ps_y = psA.tile([128, 256], F32, tag="ps")
                for u in range(2):
                    nc.tensor.matmul(ps_y[:], aoT[:, 128 * u:128 * (u + 1)],
                                     wp_t[u][:], start=(u == 0), stop=(u == 1))
                ysb = wkp.tile([128, 256], F32, tag="ysb")
                nc.vector.tensor_tensor(out=ysb[:], in0=ps_y[:], in1=bp_b[:],
                                        op=mybir.AluOpType.add)
                nc.sync.dma_start(out=yv[wt, 2 * s:2 * s + 2, :, :, :], in_=ysb[:])

    nc.compile()
    return nc


def _prep_inputs(x, w_qkv, b_qkv, w_proj, b_proj):
    x = np.asarray(x, np.float32)
    xp = np.zeros((D, H + 4, WP, C), np.float32)
    xp[:, 2:H + 2, 2:W + 2, :] = x[0]
    wq = np.ascontiguousarray(w_qkv[:, 0:C] * SCALE).astype(np.float32)
    wkk = np.ascontiguousarray(w_qkv[:, C:2 * C]).astype(np.float32)
    wv = np.ascontiguousarray(w_qkv[:, 2 * C:3 * C]).astype(np.float32)
    wp16 = np.asarray(w_proj, np.float32).astype(ml_dtypes.bfloat16)
    bq = np.asarray(b_qkv, np.float32)
    bqkv_pack = np.zeros((128, 6), np.float32)
    bqkv_pack[:, 0] = bq[0:128] * SCALE
    bqkv_pack[:, 1] = bq[128:256] * SCALE
    bqkv_pack[:, 2] = bq[256:384]
    bqkv_pack[:, 3] = bq[384:512]
    bv = np.ascontiguousarray(bq[2 * C:3 * C].reshape(1, C)).astype(np.float32)
    bp = np.ascontiguousarray(np.asarray(b_proj, np.float32).reshape(1, C))
    ident = np.eye(128, dtype=np.float32).astype(ml_dtypes.bfloat16)

    # U: key-side indicators [64, (w', d, h)] over shard voxels
    U = np.zeros((64, WP, D, HH), np.float32)
    for d in range(D):
        U[d, :, d, :] = 1.0
    for r in range(HH):
        U[4 + r, :, :, r] = 1.0
    for wpp in range(WP):
        U[12 + wpp, wpp, :, :] = 1.0
    U = U.reshape(64, NV).astype(ml_dtypes.bfloat16)

    in_maps = []
    for c in range(NCORES):
        xs = xp[:, 4 * c:4 * c + HH, :, :]         # [D, HH, WP, C] padded rows
        xT = np.ascontiguousarray(xs.reshape(NV, C).T)
        # query-ordered copy: columns in (wt, d, h own, wl) order
        xq = xs[:, 2:6, 2:2 + W, :]                # [D, 4, W, C]
        xq = xq.reshape(D, 4, NWT, 16, C).transpose(2, 0, 1, 3, 4)
        xTq = np.ascontiguousarray(xq.reshape(NQ, C).T)
        # V: query-side penalties, columns in (wt, d, hq, wl) order
        Vm = np.full((64, D, 4, W), -BIG, np.float32)
        for d in range(D):
            lo = min(max(d - 1, 0), D - KD)
            Vm[lo:lo + KD, d, :, :] = 0.0
        for hq in range(4):
            s = min(max(4 * c + hq - 2, 0), H - KH)
            for r in range(HH):
                if s <= 4 * c + r - 2 < s + KH:
                    Vm[4 + r, :, hq, :] = 0.0
        for wq_i in range(W):
            s = min(max(wq_i - 2, 0), W - KW)
            Vm[12 + s + 2:12 + s + 2 + KW, :, :, wq_i] = 0.0
        Vm = Vm.reshape(64, D, 4, NWT, 16).transpose(0, 3, 1, 2, 4)
        Vm = np.ascontiguousarray(Vm.reshape(64, NQ)).astype(ml_dtypes.bfloat16)
        in_maps.append({
            "xT": xT, "xTq": xTq, "wq": wq, "wk": wkk, "wv": wv, "wp": wp16,
            "u": U, "vq": Vm, "bqkv": bqkv_pack, "bv": bv, "bp": bp,
            "ident": ident,
        })
    return in_maps


def kernel(x, w_qkv, b_qkv, w_proj, b_proj):
    if "nc" not in _CACHE:
        _CACHE["nc"] = _build_program()
    nc = _CACHE["nc"]
    in_maps = _prep_inputs(x, w_qkv, b_qkv, w_proj, b_proj)
    res = run_bass_kernel_spmd(nc, in_maps, list(range(NCORES)))
    out = np.zeros((1, D, H, W, C), np.float32)
    for c in range(NCORES):
        y = res.results[c]["y"].reshape(NWT, D, 4, 16, C)
        y = y.transpose(1, 2, 0, 3, 4).reshape(D, 4, W, C)
        out[0, :, 4 * c:4 * c + 4, :, :] = y
    return out
